# revision 25
# baseline (speedup 1.0000x reference)
"""Trainium2 Bass kernel for nn_LNon_37460704756094 (embedding_lookup).

Math (reference):
    x   = (data - mean(data)) / std(data, ddof=1) * scalei
    s   = sigmoid(x); t = tanh(x)
    theta = th0 + th_slope * s        (affine LUT, th0=-pi, th_slope=2pi)
    velo  = v_slope * |t|             (affine LUT through 0)
    val = x * exp(velo * sin(theta)) + velo * cos(theta)
    out = (val - mean(val)) / std(val, ddof=1) * scaleo

Key identity (exact for th0=-pi, th_slope=2pi):
    theta = -pi + 2*pi*sigmoid(x) = pi * tanh(x/2)
so with u = tanh(x/2):
    sin(theta) = sin(pi*u), cos(theta) = sin(pi*u + pi/2)
and no sigmoid is needed.  The final normalization is scale invariant, so
we compute val' = val / v_slope = (x/v_slope) * exp(v_slope * |t| * sin)
+ |t| * cos, which folds all LUT slopes into activation scale immediates.

Per-core schedule (batch-sharded, 4.19M elems = [128, 32768]):
  Phase A: DMA-in f32 tiles; ACT Copy->fp16 blob (accum_out => sum(d));
           DVE TTR(blob*blob) (accum_out => sum(d^2)).  AllReduce stats,
           derive a,b on-chip.
  Phase B: 4 sections x 4 tiles, ACT table sets loaded manually:
           [silu_and_others] u=Tanh(a/2 d+b/2), t=Tanh(a d+b),
           sin=Sin(pi u), cos=Sin(pi u+pi/2); DVE |t|, p=|t|sin, q=|t|cos
           [exp_and_others]  e=Exp(v_slope p); DVE u6=(a d+b)/v_slope,
           r=u6*e, val=TTR(r+q->blob, accum sum), TTR(val^2, accum sumsq)
           AllReduce stats, derive a2,b2.
  Phase C: DVE out = a2*val+b2 (fp16) -> DMA out; host converts to f32.

Everything between the f32 HBM read and the fp16 HBM write is fp16
(validated 1.3e-3 rel err vs the f32 reference).  5 ACT passes/elem,
~9 ACT table loads total (vs 99 in the naive per-tile function order).
"""

import math

import numpy as np

import concourse.bacc as bacc
import concourse.bass as bass
import concourse.mybir as mybir
import concourse.tile as tile
from concourse.bass_utils import run_bass_kernel_spmd
from concourse.hw_specs import get_activation_tables
from concourse.tile_rust import add_dep_helper

N_CORES = 8
P = 128
B_FULL, C, H, W = 32, 64, 128, 128
PER_CORE = B_FULL // N_CORES * C * H * W          # 4,194,304
E = PER_CORE // P                                 # 32,768 elems per lane
F = 2048                                          # tile free size
NT = E // F                                       # 16 tiles
SEC_TILES = 4                                     # tiles per table-set section
NSEC = NT // SEC_TILES                            # 4 sections
N_TOTAL = B_FULL * C * H * W                      # 33,554,432

AF = mybir.ActivationFunctionType
ALU = mybir.AluOpType
AX = mybir.AxisListType
F32 = mybir.dt.float32
F16 = mybir.dt.float16
U32 = mybir.dt.uint32

LAST_RESULT = None  # BassKernelResults of the most recent run (for test.py)

_KERNEL_CACHE = {}


def _build(consts, sim_mode=False):
    """Build the SPMD Bass program. `consts` = (v_slope,)."""
    (v_slope,) = consts
    halfpi = math.pi / 2.0

    nc = bacc.Bacc(None, num_devices=N_CORES)

    tabs = get_activation_tables(nc.m.arch)
    set_names = list(tabs.keys())
    SET_SILU = set_names.index("silu_and_others")
    SET_EXP = set_names.index("exp_and_others")

    # Float activation biases are looked up in nc.const_aps; register the
    # one non-default bias we use (pi/2 for the cos pass).
    for cv in (halfpi,):
        if (F32, cv) not in nc.const_aps.aps:
            t = nc.alloc_sbuf_tensor(f"const-f32-{cv}", [P, 1], F32)
            nc.gpsimd.memset(t.ap(), cv)
            nc.const_aps.aps[(F32, cv)] = t.ap()
    nc.all_engine_barrier()

    data_in = nc.dram_tensor("data", [P, E], F32, kind="ExternalInput")
    scal_in = nc.dram_tensor("scal", [P, 2], F32, kind="ExternalInput")
    out_dram = nc.dram_tensor("out", [P, E], F16, kind="ExternalOutput")

    groups = [list(range(N_CORES))]

    def load_set(set_id):
        return nc.scalar.add_instruction(
            mybir.InstLoadActFuncSet(
                name=nc.get_next_instruction_name(),
                act_func_set_id=set_id,
                ins=[],
                outs=[],
            )
        )

    with tile.TileContext(nc) as tc:
        with (
            tc.tile_pool(name="blob", bufs=1) as blobpool,
            tc.tile_pool(name="dma", bufs=2) as dmapool,
            tc.tile_pool(name="sec", bufs=1) as secpool,
            tc.tile_pool(name="scr", bufs=2) as scr,
            tc.tile_pool(name="small", bufs=1) as smallpool,
            tc.tile_pool(name="psum", bufs=1, space="PSUM") as psumpool,
            tc.tile_pool(name="dram", bufs=1, space="DRAM") as dram,
        ):
            blobs = [
                blobpool.tile([P, F], F16, name=f"blob{j}", tag=f"blob{j}")
                for j in range(NT)
            ]
            ps = [
                secpool.tile([P, F], F16, name=f"p{k}", tag=f"p{k}")
                for k in range(SEC_TILES)
            ]
            qs = [
                secpool.tile([P, F], F16, name=f"q{k}", tag=f"q{k}")
                for k in range(SEC_TILES)
            ]
            accA1 = smallpool.tile([P, NT], F32, name="accA1", tag="accA1")
            sm = smallpool.tile([P, 32], F32, name="sm", tag="sm")
            stA = smallpool.tile([P, 2], F32, name="stA", tag="stA")
            stB = smallpool.tile([P, 2], F32, name="stB", tag="stB")
            scal_all = smallpool.tile([P, 2], F32, name="scal_all", tag="scal_all")
            ones = smallpool.tile([P, P], F32, name="ones", tag="ones")
            ones_h = smallpool.tile([P, 1], F16, name="ones_h", tag="ones_h")
            psumA = psumpool.tile([P, 2], F32, name="psumA", tag="psumA")
            psumB = psumpool.tile([P, 2], F32, name="psumB", tag="psumB")
            # column-sum accumulators for PE ones-matmul stat sums
            psA2 = psumpool.tile([1, 512], F32, name="psA2", tag="psA2")
            psB1 = psumpool.tile([1, 512], F32, name="psB1", tag="psB1")
            psB2 = psumpool.tile([1, 512], F32, name="psB2", tag="psB2")
            NCH = F // 512

            cc_a_in = dram.tile([P, 2], F32, name="cc_a_in", tag="cc_a_in")
            cc_a_out = dram.tile([P, 2], F32, name="cc_a_out", tag="cc_a_out")
            cc_b_in = dram.tile([P, 2], F32, name="cc_b_in", tag="cc_b_in")
            cc_b_out = dram.tile([P, 2], F32, name="cc_b_out", tag="cc_b_out")

            nc.gpsimd.dma_start(scal_all[:], scal_in[:])
            nc.vector.memset(ones[:], 1.0)
            nc.vector.memset(ones_h[:], 1.0)

            # ---------------- Phase A: load, fp16 copy, input stats ------
            # DMA at 2x tile granularity (2 MB transfers) for better queue
            # utilization; compute consumes the two halves.
            FD = 2 * F
            for jd in range(E // FD):
                din = dmapool.tile([P, FD], F32, name="din", tag="din")
                nc.sync.dma_start(din[:], data_in[:, jd * FD : (jd + 1) * FD])
                for h in range(2):
                    j = 2 * jd + h
                    # blob = fp16(d); accum => per-partition sum(d)
                    nc.scalar.activation(
                        blobs[j][:], din[:, h * F : (h + 1) * F], AF.Copy,
                        accum_out=accA1[:, j : j + 1],
                    )
                    # sum(d^2): square on DVE, column-sum on the idle PE
                    sqd = scr.tile([P, F], F16, name="sqd", tag="sqd")
                    nc.vector.tensor_mul(sqd[:], blobs[j][:], blobs[j][:])
                    for c in range(NCH):
                        nc.tensor.matmul(
                            psA2[:], ones_h[:], sqd[:, c * 512 : (c + 1) * 512],
                            start=(j == 0 and c == 0),
                            stop=(j == NT - 1 and c == NCH - 1),
                            skip_group_check=True,
                        )

            nc.vector.memset(stA[:, 1:2], 0.0)
            nc.vector.reduce_sum(stA[:, 0:1], accA1[:], axis=AX.X)
            nc.vector.reduce_sum(stA[0:1, 1:2], psA2[:], axis=AX.X)

            nc.gpsimd.dma_start(cc_a_in[:], stA[:])
            if sim_mode:
                nc.gpsimd.dma_start(cc_a_out[:], cc_a_in[:])
            else:
                nc.gpsimd.collective_compute(
                    "AllReduce", ALU.add, replica_groups=groups,
                    ins=[cc_a_in.opt()], outs=[cc_a_out.opt()],
                )
            nc.gpsimd.dma_start(stA[:], cc_a_out[:])
            # ones.T @ stA reduces across partitions and broadcasts
            nc.tensor.matmul(psumA[:], ones[:], stA[:])
            nc.vector.tensor_copy(sm[:, 0:2], psumA[:])

            # a = scalei / std, b = -mean * a   (std unbiased, ddof=1)
            nc.vector.tensor_scalar_mul(sm[:, 2:3], sm[:, 0:1], 1.0 / N_TOTAL)  # mean
            nc.vector.tensor_mul(sm[:, 3:4], sm[:, 0:1], sm[:, 2:3])            # S1*mean
            nc.vector.tensor_sub(sm[:, 4:5], sm[:, 1:2], sm[:, 3:4])
            nc.vector.tensor_scalar_mul(sm[:, 5:6], sm[:, 4:5], 1.0 / (N_TOTAL - 1))
            i_sqrtA = nc.scalar.activation(sm[:, 6:7], sm[:, 5:6], AF.Sqrt)     # std
            nc.vector.reciprocal(sm[:, 7:8], sm[:, 6:7])                        # 1/std
            nc.vector.tensor_mul(sm[:, 8:9], sm[:, 7:8], scal_all[:, 0:1])      # a
            nc.vector.tensor_mul(sm[:, 9:10], sm[:, 2:3], sm[:, 8:9])
            nc.vector.tensor_scalar_mul(sm[:, 10:11], sm[:, 9:10], -1.0)        # b
            nc.vector.tensor_scalar_mul(sm[:, 11:12], sm[:, 8:9], 0.5)          # a/2
            nc.vector.tensor_scalar_mul(sm[:, 12:13], sm[:, 10:11], 0.5)        # b/2
            nc.vector.tensor_scalar_mul(sm[:, 13:14], sm[:, 8:9], 1.0 / v_slope)
            nc.vector.tensor_scalar_mul(sm[:, 14:15], sm[:, 10:11], 1.0 / v_slope)
            a_ap = sm[:, 8:9]
            b_ap = sm[:, 10:11]
            aH_ap = sm[:, 11:12]
            bH_ap = sm[:, 12:13]
            a3_ap = sm[:, 13:14]
            b3_ap = sm[:, 14:15]

            # ---------------- Phase B: foilize + val stats ---------------
            # The tile scheduler reorders ACT instructions by readiness, which
            # would interleave Exp into the tanh/sin runs and force a table
            # re-load per tile.  Pin each LoadActFuncSet against the
            # neighboring ACT groups with explicit dependency edges.
            prev_exps = []
            for s in range(NSEC):
                tiles = range(s * SEC_TILES, (s + 1) * SEC_TILES)
                # stage 1: tanh/sin under one table set
                ld1 = load_set(SET_SILU)
                # section 0: run after the stats Sqrt so its sqrt-set load
                # cannot clobber the silu set
                add_dep_helper(ld1.ins, i_sqrtA.ins, reason="act set order")
                for pe_ in prev_exps:
                    add_dep_helper(ld1.ins, pe_.ins, reason="act set order")
                stage1_acts = []
                for j in tiles:
                    k = j % SEC_TILES
                    u_ = scr.tile([P, F], F16, name="u", tag="u")
                    t_ = scr.tile([P, F], F16, name="t", tag="t")
                    sin_ = scr.tile([P, F], F16, name="sin", tag="sin")
                    cos_ = scr.tile([P, F], F16, name="cos", tag="cos")
                    T1 = scr.tile([P, F], F16, name="T1", tag="T1")
                    i1 = nc.scalar.activation(u_[:], blobs[j][:], AF.Tanh,
                                              bias=bH_ap, scale=aH_ap)
                    i2 = nc.scalar.activation(t_[:], blobs[j][:], AF.Tanh,
                                              bias=b_ap, scale=a_ap)
                    add_dep_helper(i1.ins, ld1.ins, reason="act set order")
                    add_dep_helper(i2.ins, ld1.ins, reason="act set order")
                    # abs on fp16 = clear the sign bit (walrus has no abs ALU
                    # op).  Work on uint32 pairs so the packed mask is right
                    # on hardware and in CoreSim alike.  |u| is computed in
                    # place (the signed u has no other consumer).
                    nc.vector.tensor_scalar(u_[:].bitcast(U32), u_[:].bitcast(U32),
                                            0x7FFF7FFF, None, op0=ALU.bitwise_and)
                    nc.vector.tensor_scalar(T1[:].bitcast(U32), t_[:].bitcast(U32),
                                            0x7FFF7FFF, None, op0=ALU.bitwise_and)
                    # ACT Sin domain is [-pi, pi]; use |u| and sign identities:
                    # sin(pi|u|) = |sin(pi u)|, and t*|sin| = |t|*sin since
                    # sign(t) = sign(u).  cos(pi u) = sin(pi/2 - pi|u|).
                    i3 = nc.scalar.activation(sin_[:], u_[:], AF.Sin,
                                              scale=math.pi)
                    i4 = nc.scalar.activation(cos_[:], u_[:], AF.Sin,
                                              bias=halfpi, scale=-math.pi)
                    stage1_acts += [i1, i2, i3, i4]
                    nc.vector.tensor_mul(ps[k][:], t_[:], sin_[:])
                    nc.vector.tensor_mul(qs[k][:], T1[:], cos_[:])
                # stage 2: exp under the exp table set
                ld2 = load_set(SET_EXP)
                for si in stage1_acts:
                    add_dep_helper(ld2.ins, si.ins, reason="act set order")
                prev_exps = []
                for j in tiles:
                    k = j % SEC_TILES
                    e_ = scr.tile([P, F], F16, name="e", tag="e")
                    u6 = scr.tile([P, F], F16, name="u6", tag="u6")
                    r_ = scr.tile([P, F], F16, name="r", tag="r")
                    sqv = scr.tile([P, F], F16, name="sqv", tag="sqd")
                    ie = nc.scalar.activation(e_[:], ps[k][:], AF.Exp,
                                              scale=v_slope)
                    add_dep_helper(ie.ins, ld2.ins, reason="act set order")
                    prev_exps.append(ie)
                    nc.vector.tensor_scalar(u6[:], blobs[j][:], a3_ap, b3_ap,
                                            op0=ALU.mult, op1=ALU.add)
                    nc.vector.tensor_mul(r_[:], u6[:], e_[:])
                    # val -> blob (in place over d16)
                    nc.vector.tensor_add(blobs[j][:], r_[:], qs[k][:])
                    nc.vector.tensor_mul(sqv[:], blobs[j][:], blobs[j][:])
                    # sum(val), sum(val^2) column-sums on the idle PE
                    for c in range(NCH):
                        nc.tensor.matmul(
                            psB1[:], ones_h[:], blobs[j][:, c * 512 : (c + 1) * 512],
                            start=(j == 0 and c == 0),
                            stop=(j == NT - 1 and c == NCH - 1),
                            skip_group_check=True,
                        )
                    for c in range(NCH):
                        nc.tensor.matmul(
                            psB2[:], ones_h[:], sqv[:, c * 512 : (c + 1) * 512],
                            start=(j == 0 and c == 0),
                            stop=(j == NT - 1 and c == NCH - 1),
                            skip_group_check=True,
                        )

            nc.vector.memset(stB[:], 0.0)
            nc.vector.reduce_sum(stB[0:1, 0:1], psB1[:], axis=AX.X)
            nc.vector.reduce_sum(stB[0:1, 1:2], psB2[:], axis=AX.X)

            nc.gpsimd.dma_start(cc_b_in[:], stB[:])
            if sim_mode:
                nc.gpsimd.dma_start(cc_b_out[:], cc_b_in[:])
            else:
                nc.gpsimd.collective_compute(
                    "AllReduce", ALU.add, replica_groups=groups,
                    ins=[cc_b_in.opt()], outs=[cc_b_out.opt()],
                )
            nc.gpsimd.dma_start(stB[:], cc_b_out[:])
            nc.tensor.matmul(psumB[:], ones[:], stB[:])
            nc.vector.tensor_copy(sm[:, 16:18], psumB[:])

            nc.vector.tensor_scalar_mul(sm[:, 18:19], sm[:, 16:17], 1.0 / N_TOTAL)
            nc.vector.tensor_mul(sm[:, 19:20], sm[:, 16:17], sm[:, 18:19])
            nc.vector.tensor_sub(sm[:, 20:21], sm[:, 17:18], sm[:, 19:20])
            nc.vector.tensor_scalar_mul(sm[:, 21:22], sm[:, 20:21], 1.0 / (N_TOTAL - 1))
            nc.scalar.activation(sm[:, 22:23], sm[:, 21:22], AF.Sqrt)
            nc.vector.reciprocal(sm[:, 23:24], sm[:, 22:23])
            nc.vector.tensor_mul(sm[:, 24:25], sm[:, 23:24], scal_all[:, 1:2])  # a2
            nc.vector.tensor_mul(sm[:, 25:26], sm[:, 18:19], sm[:, 24:25])
            nc.vector.tensor_scalar_mul(sm[:, 26:27], sm[:, 25:26], -1.0)       # b2
            a2_ap = sm[:, 24:25]
            b2_ap = sm[:, 26:27]

            # ---------------- Phase C: normalize + store -----------------
            # Alternate across two idle scratch tags => 4 buffers in flight,
            # keeps the out-DMA queues fed.
            for j in range(NT):
                sl = slice(j * F, (j + 1) * F)
                o_ = scr.tile([P, F], F16, name="o", tag=("u" if j % 2 else "sin"))
                nc.vector.tensor_scalar(
                    o_[:], blobs[j][:], a2_ap, b2_ap, op0=ALU.mult, op1=ALU.add
                )
                nc.sync.dma_start(out_dram[:, sl], o_[:])

    nc.finalize()
    return nc


def kernel(data, params, scalei, scaleo):
    global LAST_RESULT
    data = np.ascontiguousarray(np.asarray(data, dtype=np.float32))
    params = np.asarray(params, dtype=np.float32)

    # Affine-LUT coefficients from the actual params input.  The on-chip
    # math uses theta = pi*tanh(x/2), exact iff theta_lut spans [-pi, pi].
    th_lut = params[0, 0]
    v_lut = params[1, 0]
    npts = th_lut.shape[0]
    th0 = float(th_lut[0])
    th_slope = float(th_lut[npts - 1]) - th0
    v0 = float(v_lut[0])
    v_slope = float(v_lut[npts - 1]) - v0
    assert abs(v0) < 1e-5, f"velocity LUT must start at 0 (got {v0})"
    assert abs(th0 + math.pi) < 1e-5, f"theta LUT must start at -pi (got {th0})"
    assert abs(th_slope - 2 * math.pi) < 1e-5, (
        f"theta LUT must span 2*pi (got {th_slope})"
    )

    consts = (v_slope,)
    nc = _KERNEL_CACHE.get(consts)
    if nc is None:
        nc = _build(consts)
        _KERNEL_CACHE[consts] = nc

    scal = np.tile(
        np.array(
            [[float(np.asarray(scalei).reshape(-1)[0]),
              float(np.asarray(scaleo).reshape(-1)[0])]],
            dtype=np.float32,
        ),
        (P, 1),
    )

    bpc = B_FULL // N_CORES
    in_maps = []
    for i in range(N_CORES):
        shard = np.ascontiguousarray(
            data[i * bpc : (i + 1) * bpc]
        ).reshape(P, E)
        in_maps.append({"data": shard, "scal": scal})

    res = run_bass_kernel_spmd(nc, in_maps, core_ids=list(range(N_CORES)))
    LAST_RESULT = res

    out = np.concatenate(
        [r["out"].astype(np.float32).reshape(bpc, C, H, W) for r in res.results],
        axis=0,
    )
    return out
